# revision 16
# baseline (speedup 1.0000x reference)
"""Causal self-attention (B=4, S=2048, E=1024, H=16) on 8 trn2 cores.

Sharding: tensor-parallel over heads. Core c owns heads (2c, 2c+1):
  - computes q,k,v for its 2 heads from the full x (qkv matmul with its
    128-column slice of W_attn),
  - runs causal attention for those heads,
  - multiplies by its 128-row slice of W_proj producing a PARTIAL [T, E]
    output (fp16); the host sums the 8 partials and adds b_proj.

Device layout notes (v2 — flipped AV):
  - x is passed pre-transposed (xT [E, T], bf16) so the contraction dim E
    lands on SBUF partitions for the qkv matmuls.
  - q,k are kept transposed (QT/KT [head-col, token]) which is exactly the
    lhsT/rhs layout needed for scores^T = K @ Q^T (contraction over D=64).
    Wq and bq are pre-scaled by 1/8 on the host; q/k biases are added on
    the PSUM drain (tensor_scalar_add with a [128,1] per-partition bias).
  - v is produced DIRECTLY in [token, head-col] layout (lhsT = xT chunk,
    rhs = Wv_aug) with both heads side by side plus a ones column per head
    (cols 64/129), the ones + v-bias coming from one extra matmul row
    (lhsT = ones[1,128], rhs = bvaug[1,130]). This kills the separate
    V-transpose pass of v1.
  - softmax runs on scores^T [k, q] without max-subtraction; exp output
    et [keys, q] feeds the AV matmul as lhsT in 128-query chunks:
    y[q 128, 65] += et_chunk^T @ Vaug_chunk, accumulated over key chunks
    in PSUM. Output free width is 65 (vs 512 in v1) — half the PE cycles.
    The sum row falls out as column 64/129.
  - normalization is now per-PARTITION (tokens on partitions): reciprocal
    of the sum column, then tensor_scalar_mul with a [128,1] scalar —
    no replication matmul.
  - y_n [128 tok, 128 dims] is PE-transposed once per q-chunk into the
    yTn [dims, tok] layout the projection needs; projection is unchanged.
  - the whole program is one streaming pipeline over (batch, q-block):
    qkv tiles for upcoming batches and the projection of the previous
    q-block are pumped into the attention chunk loop.
"""

import sys

if "/opt/trn_rl_repo" not in sys.path:
    sys.path.insert(0, "/opt/trn_rl_repo")

import numpy as np

N_EMBD = 1024
N_HEAD = 16
D = 64
N_CORES = 8
HPC = N_HEAD // N_CORES  # heads per core = 2
B_FULL = 4
S_FULL = 2048

USE_BF16 = True


def _patch_tile(tile):
    """This container's walrus build allows max 1 sem wait per instruction;
    stock Tile can attach several (tail drain, and any instruction whose
    inputs come from 2+ engines/queues). Split extras onto standalone
    single-wait nop carriers on the same engine, emitted just before."""
    if getattr(tile.TileContext, "_drain_split_patched", False):
        return

    orig_commit = tile.TileContext._commit_instruction

    def _commit_instruction(self, inst, lazy_reg_writes=True):
        si = inst.sync_info
        waits = list(si.on_wait) if si is not None and si.on_wait else []
        if len(waits) > 1:
            by_name = {h.name: h for h in self.sems.allocated().values()}
            for w in waits[:-1]:
                h = by_name.get(w.ant_name)
                if h is None:
                    raise RuntimeError(f"wait-split: no handle for {w.ant_name}")
                nop = self.nc.engines[inst.engine].nop(nofuse=True)
                nop.wait_op(h, w.wait_value, _wait_mode_op(w), check=False)
            inst.sync_info.on_wait = [waits[-1]]
        return orig_commit(self, inst, lazy_reg_writes)

    def _wait_mode_op(w):
        m = str(w.wait_mode)
        if "ge" in m:
            return "sem-ge"
        if "eq" in m:
            return "sem-eq"
        raise RuntimeError(f"wait-split: unsupported wait mode {m}")

    tile.TileContext._commit_instruction = _commit_instruction

    def _drain_and_barrier(self, tick_clock, wait_clock):
        nc = self.nc
        drain_inst = nc.sync.drain()
        wait_clock.add_sem_waits(
            drain_inst.ins, tile.ScopedClock({None: tick_clock.global_clock})
        )
        waits = list(drain_inst.ins.sync_info.on_wait or [])
        if len(waits) > 1:
            drain_inst.ins.sync_info.on_wait = [waits[0]]
            by_name = {}
            if self.sems is not None:
                by_name = {h.name: h for h in self.sems.allocated().values()}
            for w in waits[1:]:
                extra = nc.sync.drain()
                h = by_name.get(w.ant_name)
                if h is None:
                    raise RuntimeError(f"drain-split: no handle for {w.ant_name}")
                extra._wait_ge(h, w.wait_value)
        nc.all_engine_barrier()
        assert self.sems is not None
        popped = nc._tile_sem_poison_stack.pop()
        assert popped is self._sem_poison
        nc.clear_and_free_semaphores(list(self.sems.allocated().values()))
        nc.all_engine_barrier()

    tile.TileContext._drain_and_barrier = _drain_and_barrier
    tile.TileContext._drain_split_patched = True


def build_nc(nb=B_FULL, s=S_FULL, num_devices=N_CORES):
    import concourse.bass as bass
    import concourse.mybir as mybir
    import concourse.tile as tile
    from concourse.bass import ds, ts
    from concourse.masks import make_identity
    from collections import deque

    _patch_tile(tile)

    f32 = mybir.dt.float32
    AF = mybir.ActivationFunctionType
    E = N_EMBD
    T = nb * s
    KO = E // 128  # contraction chunks for qkv
    NT = s // 512  # token 512-tiles per batch
    NKC = s // 128  # k 128-chunks per batch
    VA = HPC * (D + 1)  # 130: v cols (both heads + ones cols)
    assert s % 512 == 0

    rdt = mybir.dt.bfloat16 if USE_BF16 else mybir.dt.float32r
    odt = mybir.dt.float16 if USE_BF16 else f32

    nc = bass.Bass(
        "TRN2", target_bir_lowering=False, debug=False, num_devices=num_devices
    )
    NBT = nb * NT
    xT = nc.dram_tensor("xT", [128, NBT, KO, 512], rdt, kind="ExternalInput")
    Wq = nc.dram_tensor("Wq", [128, KO, 128], rdt, kind="ExternalInput")
    Wk = nc.dram_tensor("Wk", [128, KO, 128], rdt, kind="ExternalInput")
    Wv = nc.dram_tensor("Wv", [128, KO, VA], rdt, kind="ExternalInput")
    bq = nc.dram_tensor("bq", [128], f32, kind="ExternalInput")
    bk = nc.dram_tensor("bk", [128], f32, kind="ExternalInput")
    bv = nc.dram_tensor("bv", [VA], rdt, kind="ExternalInput")
    Wp = nc.dram_tensor("Wp", [128, E], rdt, kind="ExternalInput")
    out = nc.dram_tensor("out", [T, E], odt, kind="ExternalOutput")

    from contextlib import ExitStack

    with tile.TileContext(nc) as tc, ExitStack() as ctx:
        const = ctx.enter_context(tc.tile_pool(name="const", bufs=1))
        pb = ctx.enter_context(tc.tile_pool(name="perb", bufs=3))
        xp = ctx.enter_context(tc.tile_pool(name="xp", bufs=3))
        mm_ps = ctx.enter_context(tc.tile_pool(name="mmps", bufs=2, space="PSUM"))
        y_ps = ctx.enter_context(tc.tile_pool(name="yps", bufs=2, space="PSUM"))
        aux_ps = ctx.enter_context(tc.tile_pool(name="auxps", bufs=2, space="PSUM"))
        expp = ctx.enter_context(tc.tile_pool(name="expp", bufs=20))
        nrm = ctx.enter_context(tc.tile_pool(name="nrm", bufs=4))
        obp = ctx.enter_context(tc.tile_pool(name="obp", bufs=4))

        # constants — Wq DMA first, then the first xt DMA so the first qkv
        # matmul's operands lead the DMA queue
        Wq_sb = const.tile([128, KO, 128], rdt, tag="wq")
        Wk_sb = const.tile([128, KO, 128], rdt, tag="wk")
        Wv_sb = const.tile([128, KO, VA], rdt, tag="wv")
        nc.sync.dma_start(Wq_sb[:], Wq[:])
        bq_sb = const.tile([128, 1], f32, tag="bq")
        bk_sb = const.tile([128, 1], f32, tag="bk")
        bv_sb = const.tile([1, VA], rdt, tag="bv")
        ones1 = const.tile([1, 128], rdt, tag="ones1")

        tiles = {}

        def alloc_batch(b):
            QT = pb.tile([128, s], rdt, tag="qt", name=f"QT{b}")
            KT = pb.tile([128, s], rdt, tag="kt", name=f"KT{b}")
            Vaug = pb.tile([128, NKC, VA], rdt, tag="vaug", name=f"Vaug{b}")
            yTn = pb.tile([128, s], rdt, tag="ytn", name=f"yTn{b}")
            tiles[b] = (QT, KT, Vaug, yTn)

        qkv_done = {b: -1 for b in range(nb)}

        def xt_fetch(b, nt):
            xt = xp.tile([128, KO, 512], rdt, tag="xt")
            nc.sync.dma_start(xt[:], xT[:, b * NT + nt])
            return xt

        def qkv_stream():
            """Single generator emitting qkv for every batch in small steps;
            the attention loop pumps it to fill PE gaps."""
            seq = [(b, nt) for b in range(nb) for nt in range(NT)]
            pending = xt_fetch(*seq[0])
            yield
            for idx, (b, nt) in enumerate(seq):
                if nt == 0:
                    alloc_batch(b)
                QT, KT, Vaug, _ = tiles[b]
                xt = pending
                if idx + 1 < len(seq):
                    pending = xt_fetch(*seq[idx + 1])
                yield
                # q and k: [dim, tok] with per-partition bias on the drain
                for Wsb, dst, bias_sb in ((Wq_sb, QT, bq_sb), (Wk_sb, KT, bk_sb)):
                    ps = aux_ps.tile([128, 512], f32, tag="aux")
                    for ko in range(KO):
                        nc.tensor.matmul(
                            ps[:],
                            lhsT=Wsb[:, ko],
                            rhs=xt[:, ko],
                            start=(ko == 0),
                            stop=(ko == KO - 1),
                        )
                        if ko % 4 == 3:
                            yield
                    nc.vector.tensor_scalar_add(
                        dst[:, ts(nt, 512)], ps[:], bias_sb[:]
                    )
                    yield
                # v: [tok, 130] per 128-token chunk; bias+ones via extra row
                for tt in range(4):
                    vp = aux_ps.tile([128, VA], f32, tag="aux")
                    for ko in range(KO):
                        nc.tensor.matmul(
                            vp[:],
                            lhsT=xt[:, ko, ds(tt * 128, 128)],
                            rhs=Wv_sb[:, ko],
                            start=(ko == 0),
                            stop=False,
                        )
                        if ko % 4 == 3:
                            yield
                    nc.tensor.matmul(
                        vp[:],
                        lhsT=ones1[:],
                        rhs=bv_sb[:],
                        start=False,
                        stop=True,
                    )
                    nc.vector.tensor_copy(Vaug[:, nt * 4 + tt], vp[:])
                    yield
                qkv_done[b] = max(qkv_done[b], nt)

        gen = qkv_stream()
        gen_live = True
        proj_q = deque()
        # kick off the first xt DMA now, then queue the remaining consts
        next(gen)
        nc.sync.dma_start(Wk_sb[:], Wk[:])
        nc.sync.dma_start(Wv_sb[:], Wv[:])
        nc.sync.dma_start(bq_sb[:], bq[:].unsqueeze(1))
        nc.sync.dma_start(bk_sb[:], bk[:].unsqueeze(1))
        nc.sync.dma_start(bv_sb[:], bv[:].unsqueeze(0))
        nc.gpsimd.memset(ones1[:], 1.0)
        Wp_sb = const.tile([128, E], rdt, tag="wp")
        nc.sync.dma_start(Wp_sb[:], Wp[:])
        ident_f32 = const.tile([128, 128], f32, tag="ident_f32")
        make_identity(nc, ident_f32[:])
        ident = const.tile([128, 128], rdt, tag="ident")
        nc.vector.tensor_copy(ident[:], ident_f32[:])

        def pump_qkv(n):
            nonlocal gen_live
            if not gen_live:
                return
            for _ in range(n):
                if next(gen, "done") == "done":
                    gen_live = False
                    break

        def pump_proj(n):
            for _ in range(n):
                while proj_q:
                    if next(proj_q[0], "done") == "done":
                        proj_q.popleft()
                    else:
                        break
                if not proj_q:
                    break

        def ensure_qkv(b, nt):
            while gen_live and qkv_done[b] < nt:
                pump_qkv(1)

        def proj_steps(b, qt):
            """Projection partial for q-block (b, qt): pumped into the NEXT
            q-block's chunk loop. Drains alternate DVE/ACT to balance; near
            the tail ACT is idle once the exp stream dries up."""
            _, _, _, yTn = tiles[b]
            base = b * s
            at_tail = b == nb - 1
            for tt in range(qt * 4, qt * 4 + 4):
                ob = obp.tile([128, E], odt, tag="ob")
                for n in range(E // 512):
                    op = aux_ps.tile([128, 512], f32, tag="aux")
                    nc.tensor.matmul(
                        op[:],
                        lhsT=yTn[:, ts(tt, 128)],
                        rhs=Wp_sb[:, ts(n, 512)],
                        start=True,
                        stop=True,
                    )
                    # ~1 in 5 drains on ACT (exp keeps ACT busy); near the
                    # tail the exp stream of the last big q-blocks keeps ACT
                    # saturated, so DVE takes everything there
                    if (not at_tail) and (tt * 2 + n) % 5 == 4:
                        nc.scalar.activation(
                            ob[:, ts(n, 512)], op[:], AF.Identity
                        )
                    else:
                        nc.vector.tensor_copy(ob[:, ts(n, 512)], op[:])
                    if n == E // 512 - 1:
                        nc.sync.dma_start(
                            out[ds(base + tt * 128, 128), :], ob[:]
                        )
                    yield

        # ---- streaming main loop over (batch, q-block) ----
        for b in range(nb):
            qts = list(range(NT)) if b + 1 < nb else list(range(NT))[::-1]
            for qt in qts:
                ensure_qkv(b, qt)
                QT, KT, Vaug, yTn = tiles[b]
                pump_qkv(2)
                nchunks = 4 * qt + 4
                # y psum: one full bank per q-chunk PAIR; region for
                # (local qc, head) at col (2*qcl+h)*65
                yps = [
                    y_ps.tile([128, 512], f32, tag="y", name=f"y{b}_{qt}_{p}")
                    for p in range(2)
                ]
                ets = []
                offs = []
                for kc in range(nchunks):
                    m = kc - 4 * qt
                    off = max(0, 128 * m)
                    w = 512 - off
                    sp = mm_ps.tile([128, 2, 512], f32, tag="s")
                    for h in range(HPC):
                        hp = h * D
                        nc.tensor.matmul(
                            sp[:, h, 0:w],
                            lhsT=KT[ds(hp, D), ts(kc, 128)],
                            rhs=QT[ds(hp, D), ds(qt * 512 + off, w)],
                            start=True,
                            stop=True,
                        )
                    et = expp.tile([128, 2, 512], rdt, tag="exp")
                    nc.scalar.activation(et[:, :, 0:w], sp[:, :, 0:w], AF.Exp)
                    if m >= 0:
                        mw = min(w, 128)
                        nc.gpsimd.affine_select(
                            et[:, :, 0:mw],
                            et[:, :, 0:mw],
                            pattern=[[0, HPC], [1, mw]],
                            compare_op=mybir.AluOpType.is_ge,
                            fill=0.0,
                            base=0,
                            channel_multiplier=-1,
                        )
                    ets.append(et)
                    offs.append(off)
                    pump_qkv(1)
                    if kc >= 3:
                        pump_proj(1)
                # flipped AV. PSUM accumulation groups are per 2KB bank
                # (one open start..stop group at a time), so each region's
                # kc-chain runs back-to-back; closed regions keep their data.
                for qcl in range(4):
                    qc4 = 4 * qt + qcl
                    for h in range(HPC):
                        for kc in range(qc4 + 1):
                            nc.tensor.matmul(
                                yps[qcl // 2][
                                    :, ds((2 * (qcl % 2) + h) * (D + 1), D + 1)
                                ],
                                lhsT=ets[kc][:, h, ds(128 * qcl - offs[kc], 128)],
                                rhs=Vaug[:, kc, ds(h * (D + 1), D + 1)],
                                start=(kc == 0),
                                stop=(kc == qc4),
                            )
                        pump_qkv(2)
                    pump_proj(1)
                    # normalize + transpose as soon as this chain closes:
                    # unblocks the y bank and keeps DVE fed while the next
                    # chain accumulates
                    yp = yps[qcl // 2]
                    cb = 2 * (qcl % 2) * (D + 1)
                    rec = nrm.tile([128, 2], f32, tag="rec")
                    nc.vector.reciprocal(rec[:], yp[:, ds(cb + D, 2, D + 1)])
                    y_n = nrm.tile([128, 128], rdt, tag="yn")
                    for h in range(HPC):
                        nc.vector.tensor_scalar_mul(
                            y_n[:, ds(h * D, D)],
                            yp[:, ds(cb + h * (D + 1), D)],
                            rec[:, ds(h, 1)],
                        )
                    tp = aux_ps.tile([128, 128], rdt, tag="aux")
                    nc.tensor.transpose(tp[:], y_n[:], ident[:])
                    nc.vector.tensor_copy(yTn[:, ts(qc4, 128)], tp[:])
                proj_q.append(proj_steps(b, qt))

        # drain remaining pumped work
        while gen_live:
            pump_qkv(8)
        while proj_q:
            pump_proj(8)

    return nc


def shard_inputs(x, W_attn, b_attn, W_proj, nb, s):
    """Build the per-core input maps (bf16 operands, f32 q/k biases)."""
    import ml_dtypes

    bf16 = ml_dtypes.bfloat16 if USE_BF16 else np.float32
    E = N_EMBD
    T = nb * s
    VA = HPC * (D + 1)
    x2d = np.asarray(x, dtype=np.float32).reshape(T, E)
    xT = np.ascontiguousarray(x2d.T).astype(bf16)
    Wa = np.asarray(W_attn, dtype=np.float32)
    Wpf = np.asarray(W_proj, dtype=np.float32)
    ba = np.asarray(b_attn, dtype=np.float32)
    KO = E // 128
    NBT = T // 512
    xTr = np.ascontiguousarray(
        xT.reshape(KO, 128, NBT, 512).transpose(1, 2, 0, 3)
    )

    def w_r(w):
        return np.ascontiguousarray(
            np.asarray(w, dtype=np.float32)
            .reshape(KO, 128, w.shape[1])
            .transpose(1, 0, 2)
        ).astype(bf16)

    in_maps = []
    for c in range(N_CORES):
        lo = c * HPC * D
        hi = lo + HPC * D
        # Wv augmented: per head 64 cols + a zero col (ones added via bias)
        Wv_aug = np.zeros((E, VA), np.float32)
        bv_aug = np.zeros((VA,), np.float32)
        for h in range(HPC):
            cl = 2 * E + lo + h * D
            Wv_aug[:, h * (D + 1) : h * (D + 1) + D] = Wa[:, cl : cl + D]
            bv_aug[h * (D + 1) : h * (D + 1) + D] = ba[2 * E + lo + h * D : 2 * E + lo + h * D + D]
            bv_aug[h * (D + 1) + D] = 1.0
        in_maps.append(
            {
                "xT": xTr,
                "Wq": w_r(Wa[:, lo:hi] * 0.125),
                "Wk": w_r(Wa[:, E + lo : E + hi]),
                "Wv": w_r(Wv_aug),
                "bq": np.ascontiguousarray(ba[lo:hi]) * 0.125,
                "bk": np.ascontiguousarray(ba[E + lo : E + hi]),
                "bv": bv_aug.astype(bf16),
                "Wp": np.ascontiguousarray(Wpf[lo:hi, :]).astype(bf16),
            }
        )
    return in_maps


_NC_CACHE = {}


def _get_nc(nb, s):
    key = (nb, s)
    if key not in _NC_CACHE:
        _NC_CACHE[key] = build_nc(nb, s)
    return _NC_CACHE[key]


def kernel(x, W_attn, b_attn, W_proj, b_proj, _trace=False):
    from concourse.bass_utils import run_bass_kernel_spmd

    nb, s, E = x.shape
    assert E == N_EMBD
    nc = _get_nc(nb, s)
    in_maps = shard_inputs(x, W_attn, b_attn, W_proj, nb, s)
    res = run_bass_kernel_spmd(nc, in_maps, list(range(N_CORES)), trace=_trace)
    acc = res.results[0]["out"].astype(np.float32)
    for c in range(1, N_CORES):
        acc += res.results[c]["out"].astype(np.float32)
    acc += np.asarray(b_proj, dtype=np.float32)
    out = acc.reshape(nb, s, E)
    kernel.last_results = res
    return out
